# revision 1
# baseline (speedup 1.0000x reference)
"""Binary-weight 3x3 conv (sign(W)), NCHW, stride 1, pad 1, on 8 trn2 cores.

Data-parallel over batch (4 images/core), implicit GEMM in fp8 DoubleRow.

Numerics: x is split host-side into x_hi = e4m3(x) and x_lo = e4m3(x - x_hi),
row-interleaved in one padded buffer. The contraction runs 14 "slots" -- all
9 hi taps + 5 residual (lo) taps -- packed two-per-DoubleRow-matmul (each DR
matmul contracts 2x128 through the fp8-interleaved PE array), so an 8-row
output block is 7 accumulating DR matmuls instead of 9 bf16 matmuls: ~1.45x
fewer PE cycles. Partial residual correction leaves L2 err
~= 0.0266*sqrt((9-5)/9) = 0.018 < 2e-2 (each corrected tap cancels 1/9 of
the e4m3 quantization error variance; weights +-1 are exact in fp8).

HW constraints honored: the DR ifmap group stride (dim-1 of the custom
4-dim access pattern) must be even and not tiny -- odd strides fault the
device -- so the 5 corrected taps are chosen to balance the offset-parity
classes and pairs are first-half x second-half within a class (deltas
58..174 elements, all even).

Schedule: per-partition DMA descriptors dominate small-transfer latency, so
the two gating transfers (first image rows + first weights) are split 4-way
by partition range across the DMA-capable engines; uninitialized-scratch
warmup matmuls bridge the input-DMA window so the HAM clock ramp (to
2.4 GHz) completes before the real stream starts; PSUM is drained to bf16
(halving store traffic, upcast on host) on alternating Vector/Scalar
engines; the final block's drain+store is split so the last DMA overlaps
the last copy.
"""

import numpy as np

import concourse.bacc as bacc
import concourse.mybir as mybir
from concourse.ap import AP
from concourse.tile import TileContext
from concourse.bass_utils import run_bass_kernel_spmd

K_CORR = 5          # residual taps corrected (of 9): L2 err ~0.0178 < 2e-2

N_CORES = 8
IMGS = 4
C = 128
O = 256
H = WD = 56
HP = WP = 58
KH = KW = 3
RB = 8
NBLK = H // RB
P = 128
N_WARM = 14

CHUNKS = [(0, 10), (8, 18), (24, 18), (40, 18)]
BLK_CHUNK = [0, 1, 1, 2, 2, 3, 3]

F32 = mybir.dt.float32
BF16 = mybir.dt.bfloat16
FP8 = mybir.dt.float8e4
NP_FP8 = mybir.dt.np(FP8)
DR = mybir.MatmulPerfMode.DoubleRow

# slot = (lvl, kh, kw); offset within an image-row block = kh*2*WP + lvl*WP + kw
def _off0(slot):
    lvl, kh, kw = slot
    return kh * 2 * WP + lvl * WP + kw


# The DoubleRow ifmap group stride must be even (16-bit granularity) and
# not tiny: pick the corrected (lo) taps so each offset-parity class has an
# even slot count, then pair first half with second half within a class
# (deltas 58..174 elements, all even).
EVEN_TAPS = [(0, 0), (0, 2), (1, 0), (1, 2), (2, 0), (2, 2)]
ODD_TAPS = [(0, 1), (1, 1), (2, 1)]
lo_o = 1 if K_CORR <= 5 else 3
lo_e = K_CORR - lo_o
lo = [(1, kh, kw) for kh, kw in EVEN_TAPS[:lo_e] + ODD_TAPS[:lo_o]]
slots = [(0, kh, kw) for kh in range(KH) for kw in range(KW)] + lo
PAIRS = []
for parity in (0, 1):
    cls = sorted((s for s in slots if _off0(s) % 2 == parity), key=_off0)
    assert len(cls) % 2 == 0, (parity, cls)
    h = len(cls) // 2
    PAIRS += list(zip(cls[:h], cls[h:]))
NPAIRS = len(PAIRS)  # 7


def _slot_off(slot, rloc):
    lvl, kh, kw = slot
    return (rloc + kh) * 2 * WP + lvl * WP + kw


def build_nc():
    nc = bacc.Bacc(None, target_bir_lowering=False)
    x = nc.dram_tensor("x", [IMGS, C, HP, 2, WP], FP8, kind="ExternalInput")
    wb = nc.dram_tensor("wb", [C, 2, NPAIRS, 2, P], FP8, kind="ExternalInput")
    out = nc.dram_tensor("out", [IMGS, O, H, WD], BF16, kind="ExternalOutput")

    with TileContext(nc) as tc:
        with (
            tc.tile_pool(name="wpool", bufs=1) as wpool,
            tc.tile_pool(name="xpool", bufs=1) as xpool,
            tc.tile_pool(name="opool", bufs=10) as opool,
            tc.tile_pool(name="psum", bufs=6, space="PSUM") as psum_pool,
        ):
            wt = wpool.tile([P, 2, NPAIRS, 2, P], FP8, name="wt")
            # warmup scratch: mostly uninitialized — the warmup matmuls only
            # exist to keep the PE busy (HAM ramp) during the input DMA and
            # their PSUM bank is never read. The 4-element memset is the
            # minimum write that lets the tile allocator place the tile.
            wsc = wpool.tile([P, 2, 448], FP8, name="wsc")
            nc.gpsimd.memset(wsc[:, 0, 0:4], 0.0)

            xts = [
                xpool.tile([P, IMGS, nr, 2, WP], FP8, name=f"xc{ci}")
                for ci, (_, nr) in enumerate(CHUNKS)
            ]

            # the first matmul gates on img0-chunk0 + wt[:,0]; a whole-tile DMA
            # has ~5us latency (128 per-partition descriptors), so split the
            # two gating transfers 4-way by partition range and issue each
            # part from a different engine so they process concurrently.
            ENGS = [nc.sync, nc.gpsimd, nc.scalar, nc.sync]
            for qi, eng in enumerate(ENGS):
                pr = slice(qi * 32, (qi + 1) * 32)
                eng.dma_start(out=xts[0][pr, 0], in_=x[0, pr, 0:CHUNKS[0][1]])
            for qi, eng in enumerate(ENGS):
                pr = slice(qi * 32, (qi + 1) * 32)
                eng.dma_start(out=wt[pr, 0], in_=wb[pr, 0])
            nc.sync.dma_start(out=xts[1][:, 0],
                              in_=x[0, :, CHUNKS[1][0]:CHUNKS[1][0] + CHUNKS[1][1]])
            for ci, (r0, nr) in list(enumerate(CHUNKS))[2:]:
                nc.sync.dma_start(out=xts[ci][:, 0], in_=x[0, :, r0:r0 + nr])
            nc.sync.dma_start(out=wt[:, 1], in_=wb[:, 1])
            for img in range(1, IMGS):
                for ci, (r0, nr) in enumerate(CHUNKS):
                    nc.sync.dma_start(out=xts[ci][:, img], in_=x[img, :, r0:r0 + nr])

            warm = psum_pool.tile([P, RB, WD], F32, name="warm", tag="pst")
            for _ in range(N_WARM):
                nc.tensor.matmul(
                    warm[:], lhsT=wsc[:, :, :P], rhs=wsc[:],
                    start=True, stop=True, perf_mode=DR,
                )

            for img in range(IMGS):
                for half in range(2):
                    for blk in range(NBLK):
                        ci = BLK_CHUNK[blk]
                        nr = CHUNKS[ci][1]
                        rloc = blk * RB - CHUNKS[ci][0]
                        base = xts[ci][:]
                        ppitch = IMGS * nr * 2 * WP
                        img_off = img * nr * 2 * WP
                        pst = psum_pool.tile([P, RB, WD], F32, name="pst",
                                             tag="pst")
                        for pi, (sa, sb) in enumerate(PAIRS):
                            offa = _slot_off(sa, rloc)
                            offb = _slot_off(sb, rloc) if sb is not None else offa
                            rhs = AP(
                                tensor=base.tensor,
                                offset=base.offset + img_off + offa,
                                ap=[[ppitch, P], [offb - offa, 2],
                                    [2 * WP, RB], [1, WD]],
                            )
                            nc.tensor.matmul(
                                pst[:],
                                lhsT=wt[:, half, pi],
                                rhs=rhs,
                                start=(pi == 0),
                                stop=(pi == NPAIRS - 1),
                                perf_mode=DR,
                            )
                        ot = opool.tile([P, RB, WD], BF16, name="ot", tag="ot")
                        last = (img == IMGS - 1 and half == 1
                                and blk == NBLK - 1)
                        if last:
                            # split the final drain so its first-half DMA
                            # overlaps the second-half copy
                            nc.vector.tensor_copy(ot[:, 0:RB // 2],
                                                  pst[:, 0:RB // 2])
                            nc.sync.dma_start(
                                out=out[img, half * P:(half + 1) * P,
                                        blk * RB:blk * RB + RB // 2, :],
                                in_=ot[:, 0:RB // 2],
                            )
                            nc.scalar.copy(out=ot[:, RB // 2:],
                                           in_=pst[:, RB // 2:])
                            nc.sync.dma_start(
                                out=out[img, half * P:(half + 1) * P,
                                        blk * RB + RB // 2:(blk + 1) * RB, :],
                                in_=ot[:, RB // 2:],
                            )
                        else:
                            if blk % 2 == 0:
                                nc.vector.tensor_copy(ot[:], pst[:])
                            else:
                                nc.scalar.copy(out=ot[:], in_=pst[:])
                            nc.sync.dma_start(
                                out=out[img, half * P:(half + 1) * P,
                                        blk * RB:(blk + 1) * RB, :],
                                in_=ot[:],
                            )
    nc.compile()
    return nc


_NC_CACHE = None


def _get_nc():
    global _NC_CACHE
    if _NC_CACHE is None:
        _NC_CACHE = build_nc()
    return _NC_CACHE


def prep_inputs(x: np.ndarray, W: np.ndarray):
    xf = np.asarray(x, dtype=np.float32)
    x_hi = xf.astype(NP_FP8)
    x_lo = (xf - x_hi.astype(np.float32)).astype(NP_FP8)
    xp = np.zeros((xf.shape[0], C, HP, 2, WP), dtype=NP_FP8)
    xp[:, :, 1:H + 1, 0, 1:WD + 1] = x_hi
    xp[:, :, 1:H + 1, 1, 1:WD + 1] = x_lo
    wsign = np.sign(np.asarray(W, dtype=np.float32)).astype(NP_FP8)
    # [O,C,3,3] -> [C, half, kh, kw, 128]
    wbt = wsign.reshape(2, P, C, KH, KW).transpose(2, 0, 3, 4, 1)
    wq = np.zeros((C, 2, NPAIRS, 2, P), dtype=NP_FP8)
    for pi, (sa, sb) in enumerate(PAIRS):
        for g, slot in enumerate((sa, sb)):
            if slot is None:
                continue
            _, kh, kw = slot
            wq[:, :, pi, g, :] = wbt[:, :, kh, kw, :]
    xs = xp.reshape(N_CORES, IMGS, C, HP, 2, WP)
    return [{"x": np.ascontiguousarray(xs[c]), "wb": wq} for c in range(N_CORES)]


def kernel(x: np.ndarray, W: np.ndarray) -> np.ndarray:
    nc = _get_nc()
    in_maps = prep_inputs(x, W)
    res = run_bass_kernel_spmd(nc, in_maps, core_ids=list(range(N_CORES)))
    outs = [res.results[c]["out"] for c in range(N_CORES)]
    return np.concatenate(outs, axis=0).astype(np.float32)



# revision 3
# speedup vs baseline: 1.0358x; 1.0358x over previous
"""Binary-weight 3x3 conv (sign(W)), NCHW, stride 1, pad 1, on 8 trn2 cores.

Data-parallel over batch (4 images/core), implicit GEMM in fp8 DoubleRow.

Numerics: x is split host-side into x_hi = e4m3(x) and x_lo = e4m3(x - x_hi),
row-interleaved in one padded buffer. The contraction runs hi taps (all 9)
plus a PARTIAL residual (lo) correction, packed two-per-DoubleRow-matmul.
The correction is ASYMMETRIC across the two output-channel halves: half 0
corrects 5 taps (center + 4 edges -> 14 slots -> 7 DR matmuls), half 1
corrects 3 taps (center + 2 edges -> 12 slots -> 6 DR matmuls). Total
13 DR matmuls per (img, row-block) instead of 14: ~7% fewer PE cycles.
Correcting interior taps first is variance-optimal (border taps multiply
zero padding on 1/56 of outputs, so they carry slightly less error).

Error model (verified to 0.5% against HW): e4m3 quantization gives rel L2
0.02667; correcting a tap of weight w_t (center 1.0, edge 55/56, corner
(55/56)^2, sum 8.787) scales the variance by (8.787 - sum w_corr)/8.787
per half. Host-sim of this scheme = 0.01969; HW adds ~0.0017 in quadrature
(bf16 output store) -> ~0.01976 < 2e-2.

HW constraints honored: the DR ifmap group stride (dim-1 of the custom
4-dim access pattern) must be even and not tiny -- odd strides fault the
device -- so each parity class (by kw, since WP is even) must have an even
slot count; pairs are first-half x second-half within a class (deltas
116/174 elements, all even).

Schedule: per-partition DMA descriptors dominate small-transfer latency, so
the gating transfers (first image rows + first two weight pairs) are split
4-way by partition range across 4 DMA-capable engines; uninitialized-scratch
warmup matmuls bridge the input-DMA window so the HAM clock ramp (to
2.4 GHz) completes before the real stream starts; PSUM is drained to bf16
(halving store traffic, upcast on host) on alternating Vector/Scalar
engines; the final block is computed as two 4-row PSUM groups so its first
drain+store overlaps the second group's matmuls.
"""

import numpy as np

import concourse.bacc as bacc
import concourse.mybir as mybir
from concourse.ap import AP
from concourse.tile import TileContext
from concourse.bass_utils import run_bass_kernel_spmd

N_CORES = 8
IMGS = 4
C = 128
O = 256
H = WD = 56
HP = WP = 58
KH = KW = 3
RB = 8
NBLK = H // RB
P = 128
N_WARM = 10

CHUNKS = [(0, 10), (8, 18), (24, 18), (40, 18)]
BLK_CHUNK = [0, 1, 1, 2, 2, 3, 3]

F32 = mybir.dt.float32
BF16 = mybir.dt.bfloat16
FP8 = mybir.dt.float8e4
NP_FP8 = mybir.dt.np(FP8)
DR = mybir.MatmulPerfMode.DoubleRow

# Residual (lo) taps corrected per output-channel half, interior-first
# (variance-optimal) while keeping each kw-parity class even-sized for
# DoubleRow pairing.
LO_TAPS = [
    [(1, 0), (1, 2), (0, 1), (1, 1), (2, 1)],  # half 0: center + 4 edges
    [(1, 0), (1, 2), (1, 1)],                  # half 1: center + 2 edges
]


# slot = (lvl, kh, kw); offset within an image-row block = kh*2*WP + lvl*WP + kw
def _off0(slot):
    lvl, kh, kw = slot
    return kh * 2 * WP + lvl * WP + kw


def _make_pairs(lo_taps):
    slots = [(0, kh, kw) for kh in range(KH) for kw in range(KW)]
    slots += [(1, kh, kw) for kh, kw in lo_taps]
    pairs = []
    for parity in (0, 1):
        cls = sorted((s for s in slots if _off0(s) % 2 == parity), key=_off0)
        assert len(cls) % 2 == 0, (parity, cls)
        h = len(cls) // 2
        pairs += list(zip(cls[:h], cls[h:]))
    for sa, sb in pairs:
        d = _off0(sb) - _off0(sa)
        assert d % 2 == 0 and d >= 58, (sa, sb, d)
    return pairs


PAIRS_BY_HALF = [_make_pairs(lt) for lt in LO_TAPS]
NPAIRS = [len(p) for p in PAIRS_BY_HALF]  # [7, 6]
TOTPAIRS = sum(NPAIRS)  # 13
GBASE = [0, NPAIRS[0]]  # global pair index base per half


def _slot_off(slot, rloc):
    lvl, kh, kw = slot
    return (rloc + kh) * 2 * WP + lvl * WP + kw


def build_nc():
    nc = bacc.Bacc(None, target_bir_lowering=False)
    x = nc.dram_tensor("x", [IMGS, C, HP, 2, WP], FP8, kind="ExternalInput")
    wb = nc.dram_tensor("wb", [C, TOTPAIRS, 2, P], FP8, kind="ExternalInput")
    out = nc.dram_tensor("out", [IMGS, O, H, WD], BF16, kind="ExternalOutput")

    with TileContext(nc) as tc:
        with (
            tc.tile_pool(name="wpool", bufs=1) as wpool,
            tc.tile_pool(name="xpool", bufs=1) as xpool,
            tc.tile_pool(name="opool", bufs=10) as opool,
            tc.tile_pool(name="psum", bufs=6, space="PSUM") as psum_pool,
        ):
            wt = wpool.tile([P, TOTPAIRS, 2, P], FP8, name="wt")
            # warmup scratch: mostly uninitialized — the warmup matmuls only
            # exist to keep the PE busy (HAM ramp) during the input DMA and
            # their PSUM bank is never read. The 4-element memset is the
            # minimum write that lets the tile allocator place the tile.
            wsc = wpool.tile([P, 2, 448], FP8, name="wsc")
            nc.gpsimd.memset(wsc[:, 0, 0:4], 0.0)

            xts = [
                xpool.tile([P, IMGS, nr, 2, WP], FP8, name=f"xc{ci}")
                for ci, (_, nr) in enumerate(CHUNKS)
            ]

            # the first matmul gates on img0-chunk0 + wt pairs 0-1; a
            # whole-tile DMA has ~5us latency (128 per-partition
            # descriptors), so split the gating transfers 4-way by partition
            # range and issue each part from a different engine so they
            # process concurrently. Non-gating weights follow on separate
            # engines/queues.
            ENGS = [nc.sync, nc.gpsimd, nc.scalar, nc.sync]
            for qi, eng in enumerate(ENGS):
                pr = slice(qi * 32, (qi + 1) * 32)
                eng.dma_start(out=xts[0][pr, 0], in_=x[0, pr, 0:CHUNKS[0][1]])
            for qi, eng in enumerate(ENGS):
                pr = slice(qi * 32, (qi + 1) * 32)
                eng.dma_start(out=wt[pr, 0:2], in_=wb[pr, 0:2])
            nc.sync.dma_start(out=xts[1][:, 0],
                              in_=x[0, :, CHUNKS[1][0]:CHUNKS[1][0] + CHUNKS[1][1]])
            nc.gpsimd.dma_start(out=wt[:, 2:NPAIRS[0]], in_=wb[:, 2:NPAIRS[0]])
            nc.scalar.dma_start(out=wt[:, NPAIRS[0]:], in_=wb[:, NPAIRS[0]:])
            for ci, (r0, nr) in list(enumerate(CHUNKS))[2:]:
                nc.sync.dma_start(out=xts[ci][:, 0], in_=x[0, :, r0:r0 + nr])
            for img in range(1, IMGS):
                for ci, (r0, nr) in enumerate(CHUNKS):
                    nc.sync.dma_start(out=xts[ci][:, img], in_=x[img, :, r0:r0 + nr])

            warm = psum_pool.tile([P, RB, WD], F32, name="warm", tag="pst")
            for _ in range(N_WARM):
                nc.tensor.matmul(
                    warm[:], lhsT=wsc[:, :, :P], rhs=wsc[:],
                    start=True, stop=True, perf_mode=DR,
                )

            for img in range(IMGS):
                for half in range(2):
                    pairs = PAIRS_BY_HALF[half]
                    gb = GBASE[half]
                    for blk in range(NBLK):
                        ci = BLK_CHUNK[blk]
                        nr = CHUNKS[ci][1]
                        rloc = blk * RB - CHUNKS[ci][0]
                        base = xts[ci][:]
                        ppitch = IMGS * nr * 2 * WP
                        img_off = img * nr * 2 * WP
                        last = (img == IMGS - 1 and half == 1
                                and blk == NBLK - 1)
                        # the final block runs as two 4-row PSUM groups so
                        # the first drain+store overlaps the second group's
                        # matmuls, shortening the kernel tail.
                        subs = (0, 1) if last else (0,)
                        rows = RB // 2 if last else RB
                        for sub in subs:
                            r0b = rloc + sub * rows
                            pst = psum_pool.tile([P, rows, WD], F32,
                                                 name="pst", tag="pst")
                            for pi, (sa, sb) in enumerate(pairs):
                                offa = _slot_off(sa, r0b)
                                offb = _slot_off(sb, r0b)
                                rhs = AP(
                                    tensor=base.tensor,
                                    offset=base.offset + img_off + offa,
                                    ap=[[ppitch, P], [offb - offa, 2],
                                        [2 * WP, rows], [1, WD]],
                                )
                                nc.tensor.matmul(
                                    pst[:],
                                    lhsT=wt[:, gb + pi],
                                    rhs=rhs,
                                    start=(pi == 0),
                                    stop=(pi == len(pairs) - 1),
                                    perf_mode=DR,
                                )
                            ot = opool.tile([P, rows, WD], BF16, name="ot",
                                            tag="ot")
                            orow = blk * RB + sub * rows
                            if last:
                                if sub == 0:
                                    nc.vector.tensor_copy(ot[:], pst[:])
                                    nc.sync.dma_start(
                                        out=out[img, half * P:(half + 1) * P,
                                                orow:orow + rows, :],
                                        in_=ot[:],
                                    )
                                else:
                                    # final drain: split copy and store
                                    # across engines/queues so descriptor
                                    # writes proceed in parallel
                                    nc.scalar.copy(out=ot[:, 0:rows // 2],
                                                   in_=pst[:, 0:rows // 2])
                                    nc.gpsimd.dma_start(
                                        out=out[img, half * P:(half + 1) * P,
                                                orow:orow + rows // 2, :],
                                        in_=ot[:, 0:rows // 2],
                                    )
                                    nc.vector.tensor_copy(ot[:, rows // 2:],
                                                          pst[:, rows // 2:])
                                    nc.sync.dma_start(
                                        out=out[img, half * P:(half + 1) * P,
                                                orow + rows // 2:orow + rows,
                                                :],
                                        in_=ot[:, rows // 2:],
                                    )
                            else:
                                if blk % 2 == 0:
                                    nc.vector.tensor_copy(ot[:], pst[:])
                                else:
                                    nc.scalar.copy(out=ot[:], in_=pst[:])
                                nc.sync.dma_start(
                                    out=out[img, half * P:(half + 1) * P,
                                            orow:orow + rows, :],
                                    in_=ot[:],
                                )
    nc.compile()
    return nc


_NC_CACHE = None


def _get_nc():
    global _NC_CACHE
    if _NC_CACHE is None:
        _NC_CACHE = build_nc()
    return _NC_CACHE


def prep_inputs(x: np.ndarray, W: np.ndarray):
    xf = np.asarray(x, dtype=np.float32)
    x_hi = xf.astype(NP_FP8)
    x_lo = (xf - x_hi.astype(np.float32)).astype(NP_FP8)
    xp = np.zeros((xf.shape[0], C, HP, 2, WP), dtype=NP_FP8)
    xp[:, :, 1:H + 1, 0, 1:WD + 1] = x_hi
    xp[:, :, 1:H + 1, 1, 1:WD + 1] = x_lo
    wsign = np.sign(np.asarray(W, dtype=np.float32)).astype(NP_FP8)
    # [O,C,3,3] -> [C, half, kh, kw, 128]
    wbt = wsign.reshape(2, P, C, KH, KW).transpose(2, 0, 3, 4, 1)
    wq = np.zeros((C, TOTPAIRS, 2, P), dtype=NP_FP8)
    for half in range(2):
        for pi, (sa, sb) in enumerate(PAIRS_BY_HALF[half]):
            for g, slot in enumerate((sa, sb)):
                _, kh, kw = slot
                wq[:, GBASE[half] + pi, g, :] = wbt[:, half, kh, kw, :]
    xs = xp.reshape(N_CORES, IMGS, C, HP, 2, WP)
    return [{"x": np.ascontiguousarray(xs[c]), "wb": wq} for c in range(N_CORES)]


def kernel(x: np.ndarray, W: np.ndarray) -> np.ndarray:
    nc = _get_nc()
    in_maps = prep_inputs(x, W)
    res = run_bass_kernel_spmd(nc, in_maps, core_ids=list(range(N_CORES)))
    outs = [res.results[c]["out"] for c in range(N_CORES)]
    return np.concatenate(outs, axis=0).astype(np.float32)
